# revision 8
# baseline (speedup 1.0000x reference)
"""Trainium2 Bass kernel for multi-head attention (GQA + RoPE + causal).

Problem shapes (hardcoded):
  x: (2, 2048, 2048)  Wq: (2048, 2048->512/core)  Wk/Wv: (2048, 512->128/core)
  Wo: (2048->512/core, 2048)  cos/sin: (2048, 64)  mask: causal (1,1,2048,2048)

Sharding: 8 cores = 2 batches (DP) x 4 head groups (TP).  Each core handles
one batch sample and 8 query heads (= 2 KV heads).  Wo's input dim is
sharded, so each core produces a partial (2048, 2048) fp16 output; the host
sums the 4 partials per batch in fp32.

V2 design notes (vs the v1 baseline at 448us):
  - The scalar (ACT) engine runs ONLY the 160 softmax exps.  All DMA issues,
    PSUM drains and normalization copies that clogged its queue (v1's first
    exp started at 136us) are moved to sync/vector/gpsimd.
  - RoPE's rotate-half is a vector-engine STREAM_SHUFFLE instead of two
    SBUF-to-SBUF DMAs: head_dim is host-permuted ([0:16,32:48,16:32,48:64]
    per head in Wq/Wk/cos/sin) so the +-32 pair swap becomes an intra-
    quadrant rotation by 16 partitions (one mask for all four quadrants).
  - K is projected directly in GQA-duplicated layout (host duplicates the
    per-kv-head Wk columns), removing the kdup SBUF-to-SBUF DMAs.
  - All weights/x are host-packed so each input is ONE contiguous DMA.
  - Diagonal score tiles are narrowed to their unmasked column range
    (512-128i), cutting exp and matmul work by ~15%.
  - Emission interleaves phase-1 QKV chains (2-PSUM-bank accumulation
    chains), attention k-tile units, and output-projection units so the PE
    queue always has ready work while exp runs: per k-tile the PE has
    ~860ns of matmul vs ~1150ns of exp, and the deficit is filled with
    chain/oproj matmuls, keeping the PE continuously busy (p-state ramp).
  PSUM budget: acc(1 bank x2) + stt(2 banks x2) + pv(1 bank x2) = 8 banks.
"""

import os
import sys
from collections import deque

import numpy as np

if "/opt/trn_rl_repo" not in sys.path:
    sys.path.insert(0, "/opt/trn_rl_repo")

SEQ = 2048
DIM = 2048
HEAD_DIM = 64
N_HEADS_CORE = 8  # query heads per core
DQ = N_HEADS_CORE * HEAD_DIM  # 512
SCALE = HEAD_DIM ** -0.5
N_CORES = 8
NSEQT = SEQ // 128  # 16 k-tiles / s-tiles
NQB = SEQ // 512  # 4 q/seq blocks
NDIN = DIM // 128  # 16 contraction chunks

# RoPE head-dim permutation: position p holds dim PERM64[p]; pairs (d, d+32)
# land 16 partitions apart inside one 32-partition quadrant.
PERM64 = (list(range(0, 16)) + list(range(32, 48))
          + list(range(16, 32)) + list(range(48, 64)))
SHUF = list(range(16, 32)) + list(range(0, 16))  # rotate-by-16 per quadrant

_PROGRAM_CACHE = {}


def _build_program(causal: bool):
    import concourse.bass as bass  # noqa: F401
    import concourse.mybir as mybir
    from concourse import bacc
    from concourse.masks import make_identity
    from concourse.tile import TileContext

    f32 = mybir.dt.float32
    f16 = mybir.dt.float16
    AOT = mybir.AluOpType
    EXPF = mybir.ActivationFunctionType.Exp

    MM = lambda w: (6.0 + w) / 2.4  # noqa: E731  PE matmul ns at full clock
    EXP = lambda n: (n + 352.0) / 1.2  # noqa: E731  ACT exp ns

    nc = bacc.Bacc(None, target_bir_lowering=False)
    xt4 = nc.declare_dram_parameter("xt4", [128, NDIN, SEQ], f16, isOutput=False)
    wqp = nc.declare_dram_parameter("wqp", [128, NDIN, DQ], f16, isOutput=False)
    wk0 = nc.declare_dram_parameter("wk0", [128, NDIN, 128], f16, isOutput=False)
    wk1 = nc.declare_dram_parameter("wk1", [128, NDIN, 128], f16, isOutput=False)
    wvp = nc.declare_dram_parameter("wvp", [128, NDIN, 128], f16, isOutput=False)
    wop = nc.declare_dram_parameter("wop", [128, 4, DIM], f16, isOutput=False)
    cos2 = nc.declare_dram_parameter("cos2", [128, SEQ], f16, isOutput=False)
    sin2 = nc.declare_dram_parameter("sin2", [128, SEQ], f16, isOutput=False)
    out = nc.declare_dram_parameter("out", [SEQ, DIM], f16, isOutput=True)

    with TileContext(nc) as tc:
        with tc.tile_pool(name="pa", bufs=1) as pa, \
             tc.tile_pool(name="pw", bufs=1) as pw, \
             tc.tile_pool(name="pp", bufs=1, space="PSUM") as pp:

            # ---------------- persistent SBUF tiles ----------------
            wq_sb = pa.tile([128, NDIN, DQ], f16, name="wq_sb", tag="wq_sb")
            wk_sb = [pa.tile([128, NDIN, 128], f16, name=f"wk_sb{g}",
                             tag=f"wk_sb{g}") for g in range(2)]
            wv_sb = pa.tile([128, NDIN, 128], f16, name="wv_sb", tag="wv_sb")
            wo_sb = pa.tile([128, 4, DIM], f16, name="wo_sb", tag="wo_sb")
            cos_sb = pa.tile([128, SEQ], f16, name="cos_sb", tag="cos_sb")
            sin_sb = pa.tile([128, SEQ], f16, name="sin_sb", tag="sin_sb")
            qt = [[pa.tile([128, 512], f16, name=f"qt{t}_{j}",
                           tag=f"qt{t}_{j}") for j in range(NQB)]
                  for t in range(4)]
            kd = [[pa.tile([128, 512], f16, name=f"kd{g}_{j}",
                           tag=f"kd{g}_{j}") for j in range(NQB)]
                  for g in range(2)]
            vtiles = [pa.tile([128, 130], f16, name=f"vt{i}", tag=f"vt{i}")
                      for i in range(NSEQT)]
            attnT = [[pa.tile([128, 512], f16, name=f"attnT{t}_{j}",
                              tag=f"attnT{t}_{j}") for j in range(NQB)]
                     for t in range(4)]
            vtr = [pa.tile([128, 512], f16, name=f"vtr{j}", tag=f"vtr{j}")
                   for j in range(NQB)]
            identity = pa.tile([128, 128], f16, name="identity",
                               tag="identity")
            scratch = pa.tile([1, 2], f32, name="scratch", tag="scratch")

            # ---------------- prologue ----------------
            make_identity(nc, identity)
            nc.sync.dma_start(out=cos_sb, in_=cos2[:, :])
            nc.sync.dma_start(out=sin_sb, in_=sin2[:, :])
            nc.sync.dma_start(out=wq_sb, in_=wqp[:, :, :])

            xts = {}

            def prefetch(j):
                t = pw.tile([128, NDIN, 512], f16, name="xt", tag="xt",
                            bufs=3)
                xts[j] = t
                nc.sync.dma_start(out=t, in_=xt4[:, :, j * 512:(j + 1) * 512])

            prefetch(0)
            nc.sync.dma_start(out=wk_sb[0], in_=wk0[:, :, :])
            nc.sync.dma_start(out=wk_sb[1], in_=wk1[:, :, :])
            nc.sync.dma_start(out=wv_sb, in_=wvp[:, :, :])
            nc.sync.dma_start(out=wo_sb, in_=wop[:, :, :])
            prefetch(1)
            # warm the ACT exp table during phase 1
            nc.scalar.activation(out=scratch, in_=identity[0:1, 0:2],
                                 func=EXPF)
            for i in range(NSEQT):
                nc.vector.memset(vtiles[i][:, 64:65], 1.0)
                nc.vector.memset(vtiles[i][:, 129:130], 1.0)

            # ---------------- unit emitters ----------------
            CHAIN_DEFS = [("q", 0), ("q", 1), ("q", 2), ("q", 3),
                          ("k", 0), ("k", 1), ("v", 0)]
            accs = {}
            pts = {}
            pvs_tiles = {}
            ostages = {}

            def rope(chunk, j):
                sl = slice(j * 512, (j + 1) * 512)
                rot = pw.tile([128, 512], f16, name="rot", tag="rot", bufs=3)
                nc.vector.stream_shuffle(out=rot, in_=chunk, mask=SHUF)
                nc.gpsimd.tensor_tensor(out=rot, in0=rot, in1=sin_sb[:, sl],
                                        op=AOT.mult)
                nc.vector.tensor_tensor(out=chunk, in0=chunk,
                                        in1=cos_sb[:, sl], op=AOT.mult)
                nc.vector.tensor_add(out=chunk, in0=chunk, in1=rot)

            def kt_list(j):
                if causal:
                    return ([(4 * j + i, 128 * i, 512 - 128 * i)
                             for i in range(4)]
                            + [(kt, 0, 512) for kt in range(0, 4 * j)])
                return [(kt, 0, 512) for kt in range(NSEQT)]

            def emit(u):
                kindu = u[0]
                if kindu == "cmm":
                    _, j, ci, c = u
                    kind, idx = CHAIN_DEFS[ci]
                    if c == 0:
                        accs[(j, ci)] = pp.tile([128, 512], f32, name="acc",
                                                tag="acc", bufs=2)
                    acc = accs[(j, ci)]
                    if kind == "q":
                        lhsT = wq_sb[:, c, idx * 128:(idx + 1) * 128]
                    elif kind == "k":
                        lhsT = wk_sb[idx][:, c, :]
                    else:
                        lhsT = wv_sb[:, c, :]
                    nc.tensor.matmul(acc, lhsT=lhsT, rhs=xts[j][:, c, :],
                                     start=(c == 0), stop=(c == NDIN - 1))
                    return MM(512), 0.0
                if kindu == "cdrain":
                    _, j, ci = u
                    kind, idx = CHAIN_DEFS[ci]
                    acc = accs.pop((j, ci))
                    if kind == "q":
                        nc.vector.tensor_copy(out=qt[idx][j], in_=acc)
                        rope(qt[idx][j], j)
                    elif kind == "k":
                        nc.vector.tensor_copy(out=kd[idx][j], in_=acc)
                        rope(kd[idx][j], j)
                    else:
                        nc.vector.tensor_copy(out=vtr[j], in_=acc)
                    return 0.0, 0.0
                if kindu == "vtrans":
                    _, j, i2 = u
                    tp = pp.tile([128, 128], f16, name="vt_ps", tag="acc",
                                 bufs=2)
                    nc.tensor.transpose(tp,
                                        vtr[j][:, i2 * 128:(i2 + 1) * 128],
                                        identity)
                    i = 4 * j + i2
                    nc.vector.tensor_copy(out=vtiles[i][:, 0:64],
                                          in_=tp[:, 0:64])
                    nc.vector.tensor_copy(out=vtiles[i][:, 65:129],
                                          in_=tp[:, 64:128])
                    return MM(128), 0.0
                if kindu == "S":
                    _, j, hp, i, kts = u
                    kt, o, w = kts[i]
                    g = hp // 2
                    lk = kd[g][kt // 4]
                    ck = slice((kt % 4) * 128, (kt % 4 + 1) * 128)
                    stt = pp.tile([128, 2, 512], f32, name="stt", tag="stt",
                                  bufs=2)
                    nc.tensor.matmul(stt[:, 0, o:512], lhsT=lk[0:64, ck],
                                     rhs=qt[hp][j][0:64, o:512],
                                     start=True, stop=True,
                                     tile_position=(0, 0))
                    nc.tensor.matmul(stt[:, 1, o:512], lhsT=lk[64:128, ck],
                                     rhs=qt[hp][j][64:128, o:512],
                                     start=True, stop=True,
                                     tile_position=(64, 0))
                    pt = pw.tile([128, 2, 512], f16, name="pt", tag="pt",
                                 bufs=6)
                    nc.scalar.activation(out=pt[:, :, o:512],
                                         in_=stt[:, :, o:512],
                                         func=EXPF, scale=SCALE)
                    if causal and kt >= 4 * j:
                        nc.gpsimd.affine_select(
                            out=pt[:, :, o:512], in_=pt[:, :, o:512],
                            pattern=[[0, 2], [1, w]],
                            compare_op=AOT.is_ge,
                            fill=0.0, base=0, channel_multiplier=-1)
                    pts[(j, hp, i)] = pt
                    return 2 * MM(w), EXP(2 * w)
                if kindu == "PV":
                    _, j, hp, i, kts = u
                    kt, o, w = kts[i]
                    g = hp // 2
                    if i == 0:
                        pvs_tiles[(j, hp)] = (
                            pp.tile([65, 512], f32, name="pv_e", tag="pv",
                                    bufs=2),
                            pp.tile([65, 512], f32, name="pv_o", tag="pv",
                                    bufs=2))
                    pv_e, pv_o = pvs_tiles[(j, hp)]
                    pt = pts.pop((j, hp, i))
                    st, sp = (i == 0), (i == len(kts) - 1)
                    lv = vtiles[kt][:, 65 * g:65 * g + 65]
                    nc.tensor.matmul(pv_e[:, o:512], lhsT=lv,
                                     rhs=pt[:, 0, o:512], start=st, stop=sp)
                    nc.tensor.matmul(pv_o[:, o:512], lhsT=lv,
                                     rhs=pt[:, 1, o:512], start=st, stop=sp)
                    return 2 * MM(w), 0.0
                if kindu == "NORM":
                    _, j, hp = u
                    pv_pair = pvs_tiles.pop((j, hp))
                    for par in range(2):
                        pv = pv_pair[par]
                        pvs = pw.tile([64, 512], f32, name="pvs", tag="pvs",
                                      bufs=4)
                        nc.vector.tensor_copy(out=pvs, in_=pv[0:64, :])
                        den = pw.tile([1, 512], f32, name="den", tag="den",
                                      bufs=4)
                        nc.scalar.copy(out=den, in_=pv[64:65, :])
                        rec = pw.tile([1, 512], f32, name="rec", tag="rec",
                                      bufs=4)
                        nc.vector.reciprocal_approx_fast(out=rec, in_=den)
                        rbc = pw.tile([64, 512], f32, name="rbc", tag="rbc",
                                      bufs=4)
                        nc.gpsimd.partition_broadcast(out_ap=rbc, in_ap=rec)
                        nc.vector.tensor_tensor(
                            out=attnT[hp][j][64 * par:64 * par + 64, :],
                            in0=pvs, in1=rbc, op=AOT.mult)
                    return 0.0, 2 * EXP(512)
                if kindu == "oproj":
                    _, j, si, dm = u
                    s_ = 4 * j + si
                    if dm == 0:
                        ostages[(j, si)] = pw.tile([128, DIM], f16,
                                                   name="ostage",
                                                   tag="ostage", bufs=2)
                    ost = ostages[(j, si)]
                    ops = pp.tile([128, 512], f32, name="acc", tag="acc",
                                  bufs=2)
                    for c in range(4):
                        nc.tensor.matmul(
                            ops,
                            lhsT=attnT[c][j][:, si * 128:(si + 1) * 128],
                            rhs=wo_sb[:, c, dm * 512:(dm + 1) * 512],
                            start=(c == 0), stop=(c == 3))
                    if dm % 2 == 0:
                        nc.vector.tensor_copy(
                            out=ost[:, dm * 512:(dm + 1) * 512], in_=ops)
                    else:
                        nc.scalar.copy(
                            out=ost[:, dm * 512:(dm + 1) * 512], in_=ops)
                    if dm == 3:
                        nc.sync.dma_start(
                            out=out[s_ * 128:(s_ + 1) * 128, :],
                            in_=ostages.pop((j, si)))
                    return 4 * MM(512), 0.0
                raise AssertionError(kindu)

            def chain_units(j):
                units = []
                for ci, (kind, _) in enumerate(CHAIN_DEFS):
                    units += [("cmm", j, ci, c) for c in range(NDIN)]
                    units.append(("cdrain", j, ci))
                    if kind == "v":
                        units += [("vtrans", j, i2) for i2 in range(4)]
                return units

            def attn_units(j):
                units = []
                kts = kt_list(j)
                n = len(kts)
                for hp in range(4):
                    for i in range(n):
                        units.append(("S", j, hp, i, kts))
                        if i >= 1:
                            units.append(("PV", j, hp, i - 1, kts))
                    units.append(("PV", j, hp, n - 1, kts))
                    units.append(("NORM", j, hp))
                return units

            def oproj_units(j):
                return [("oproj", j, si, dm)
                        for si in range(4) for dm in range(4)]

            def run_seg(spine, fillers):
                deficit = 0.0
                fq = deque(fillers)
                for u in spine:
                    pe, act = emit(u)
                    deficit = max(deficit + act - pe, -2000.0)
                    while deficit > 100.0 and fq:
                        pe_f, _ = emit(fq.popleft())
                        deficit -= pe_f
                for u in fq:
                    emit(u)

            # ---------------- schedule ----------------
            run_seg([], chain_units(0))
            prefetch(2)
            run_seg(attn_units(0), chain_units(1))
            prefetch(3)
            run_seg(attn_units(1), oproj_units(0) + chain_units(2))
            run_seg(attn_units(2), chain_units(3))
            run_seg(attn_units(3), oproj_units(1) + oproj_units(2))
            run_seg([], oproj_units(3))

    nc.compile()
    return nc


def _get_program(causal: bool):
    key = ("v2", causal)
    if key not in _PROGRAM_CACHE:
        _PROGRAM_CACHE[key] = _build_program(causal)
    return _PROGRAM_CACHE[key]


def _check_causal(mask: np.ndarray) -> bool:
    m = mask.reshape(SEQ, SEQ)
    idx = np.array([0, 1, 7, 100, 1000, 2047])
    sub = m[np.ix_(idx, idx)]
    expect_zero = idx[:, None] >= idx[None, :]
    if not np.all(sub[expect_zero] == 0.0):
        return False
    if not np.all(sub[~expect_zero] < -1e30):
        return False
    return True


def _core_inputs(x, Wq, Wk, Wv, Wo, cos2, sin2, b, g4):
    """Host-side packing for one core (batch b, head-group g4)."""
    f16 = np.float16
    # x^T packed [128, 16, 2048] so each j-block is one strided DMA
    xT = x[b].T.astype(f16)  # (2048, 2048)
    xt4 = np.ascontiguousarray(
        xT.reshape(NDIN, 128, SEQ).transpose(1, 0, 2))
    # Wq slice with per-head RoPE permutation, packed [128, 16, 512]
    wq_c = Wq[:, g4 * DQ:(g4 + 1) * DQ]
    col_perm = np.concatenate(
        [h * 64 + np.asarray(PERM64) for h in range(N_HEADS_CORE)])
    wq_p = wq_c[:, col_perm].astype(f16)
    wqp = np.ascontiguousarray(
        wq_p.reshape(NDIN, 128, DQ).transpose(1, 0, 2))
    # Wk per kv head, permuted and duplicated to both partition halves
    wk_c = Wk[:, g4 * 128:(g4 + 1) * 128]
    p64 = np.asarray(PERM64)
    k0 = wk_c[:, p64].astype(f16)
    k1 = wk_c[:, 64 + p64].astype(f16)
    wk0 = np.ascontiguousarray(
        np.concatenate([k0, k0], axis=1).reshape(NDIN, 128, 128)
        .transpose(1, 0, 2))
    wk1 = np.ascontiguousarray(
        np.concatenate([k1, k1], axis=1).reshape(NDIN, 128, 128)
        .transpose(1, 0, 2))
    wvp = np.ascontiguousarray(
        Wv[:, g4 * 128:(g4 + 1) * 128].astype(f16)
        .reshape(NDIN, 128, 128).transpose(1, 0, 2))
    wop = np.ascontiguousarray(
        Wo[g4 * DQ:(g4 + 1) * DQ, :].astype(f16)
        .reshape(4, 128, DIM).transpose(1, 0, 2))
    return {"xt4": xt4, "wqp": wqp, "wk0": wk0, "wk1": wk1, "wvp": wvp,
            "wop": wop, "cos2": cos2, "sin2": sin2}


def _rope_tables(cos, sin):
    p64 = np.asarray(PERM64)
    cosT = cos.T[p64]  # (64, SEQ), permuted rows
    sinT = sin.T[p64]
    sign = np.where((np.arange(64) % 32) < 16, -1.0, 1.0)[:, None]
    sin_signed = sign * sinT
    cos2 = np.ascontiguousarray(np.tile(cosT, (2, 1))).astype(np.float16)
    sin2 = np.ascontiguousarray(np.tile(sin_signed, (2, 1))).astype(np.float16)
    return cos2, sin2


def kernel(x, Wq, Wk, Wv, Wo, cos, sin, attention_mask):
    from concourse.bass_utils import run_bass_kernel_spmd

    x = np.asarray(x, dtype=np.float32)
    Wq = np.asarray(Wq, dtype=np.float32)
    Wk = np.asarray(Wk, dtype=np.float32)
    Wv = np.asarray(Wv, dtype=np.float32)
    Wo = np.asarray(Wo, dtype=np.float32)
    cos = np.asarray(cos, dtype=np.float32)
    sin = np.asarray(sin, dtype=np.float32)
    mask = np.asarray(attention_mask, dtype=np.float32)

    causal = _check_causal(mask)
    if not causal:
        assert np.all(mask == 0.0), (
            "kernel only supports the causal or all-zero attention masks")

    cos2, sin2 = _rope_tables(cos, sin)
    nc = _get_program(causal)

    in_maps = []
    for core in range(N_CORES):
        b, g4 = core // 4, core % 4
        in_maps.append(_core_inputs(x, Wq, Wk, Wv, Wo, cos2, sin2, b, g4))

    trace = bool(int(os.environ.get("KERNEL_TRACE", "0")))
    res = run_bass_kernel_spmd(nc, in_maps, list(range(N_CORES)), trace=trace)
    if trace:
        kernel.last_exec_time_ns = res.exec_time_ns
        kernel.last_profile = res.profile_json

    outs = [res.results[i]["out"].astype(np.float32) for i in range(N_CORES)]
    y0 = outs[0] + outs[1] + outs[2] + outs[3]
    y1 = outs[4] + outs[5] + outs[6] + outs[7]
    return np.stack([y0, y1])
